# revision 5
# baseline (speedup 1.0000x reference)
"""Trainium2 Bass kernel for the CNN-attention module.

Per batch element b:
    feats   = cnn[b].reshape(C, P).T            # [P, C], P = 14*14
    att     = tanh(feats @ W_cnn + hidden[b] @ W_hid + b_att)   # [P, ATT]
    scores  = att @ W_score (+ b_score, irrelevant under softmax)
    weights = softmax(scores)                   # [P]
    context = weights @ feats                   # [C]

Sharding: data-parallel over batch, 16 batches per core on 8 cores.

Per-core layout (batch pairs, "bg" = group of BG=2 batches, N=392 columns):
  GEMM1 (dominant): att_pre[a, (b,p)] = sum_c W_cnn[c,a] * cnn[b,c,p]
    - stationary lhsT = W_cnn chunk [128c, 128a], moving rhs = cnn natural
      layout [128c, 392], accumulated over 16 c-chunks in PSUM, float32r
      (full PE rate at N>=256, fp32 data in SBUF).
  tanh on ScalarE straight out of PSUM with per-partition bias
      (b_att + hidden@W_hid precomputed per (a,b)).
  scores: M=1 matmuls (lhsT = W_score chunk [128,1]), accumulated in PSUM.
  softmax: exp+sum fused in one ScalarE activation per batch (accum_out);
      b_score and max-subtraction drop out (shift invariance; scores are
      O(1) so exp is safe in fp32).
  weights broadcast to 128 partitions via a K=1 PE matmul with a ones
      column (w_bcast = ones[128,1] @ w[1,392]).
  context: DVE elementwise mul (cnn * w_bcast) + ScalarE accumulate-copy
      per batch (activation Copy with accum_out) -> context columns.

The per-bg tail (softmax/broadcast/context) is software-pipelined one bg
behind GEMM so the PE stream never waits on the ScalarE/DVE chain.
"""

from contextlib import ExitStack

import numpy as np

B = 128
C = 2048
HW = 196  # 14*14
ATT = 512
HID = 512
N_CORES = 8
B_CORE = B // N_CORES  # 16
BG = 2  # batches per group
NBG = B_CORE // BG  # 8
COLS = BG * HW  # 392
NCC = C // 128  # 16
NAC = ATT // 128  # 4
NHC = HID // 128  # 4


def _emit(ctx: ExitStack, tc, cnn, hid, watt, batt, wsc, ctx_out, w_out):
    import concourse.bass as bass  # noqa: F401
    from concourse import mybir

    nc = tc.nc
    f32 = mybir.dt.float32
    f32r = mybir.dt.float32r
    AF = mybir.ActivationFunctionType

    singles = ctx.enter_context(tc.tile_pool(name="singles", bufs=1))
    cnn_pool = ctx.enter_context(tc.tile_pool(name="cnn", bufs=4))
    att_pool = ctx.enter_context(tc.tile_pool(name="att", bufs=2))
    soft_pool = ctx.enter_context(tc.tile_pool(name="soft", bufs=3))
    tmp_pool = ctx.enter_context(tc.tile_pool(name="tmp", bufs=3))
    junk_pool = ctx.enter_context(tc.tile_pool(name="junk", bufs=2))
    ctxc_pool = ctx.enter_context(tc.tile_pool(name="ctxc", bufs=2))
    ps_att = ctx.enter_context(tc.tile_pool(name="ps_att", bufs=3, space="PSUM"))
    ps_sc = ctx.enter_context(tc.tile_pool(name="ps_sc", bufs=2, space="PSUM"))
    ps_wb = ctx.enter_context(tc.tile_pool(name="ps_wb", bufs=2, space="PSUM"))
    ps_hb = ctx.enter_context(tc.tile_pool(name="ps_hb", bufs=1, space="PSUM"))

    # ---- setup: weights into SBUF ----
    w_sb = singles.tile([128, NCC, ATT], f32r)
    nc.sync.dma_start(
        out=w_sb, in_=watt[0:C].rearrange("(cc part) a -> part cc a", part=128)
    )
    whid_sb = singles.tile([128, NHC, ATT], f32r)
    nc.sync.dma_start(
        out=whid_sb, in_=watt[C : C + HID].rearrange("(hc part) a -> part hc a", part=128)
    )
    hidt_sb = singles.tile([128, NHC, B_CORE], f32r)
    hid_t = hid.rearrange("b (hc part) -> part hc b", part=128)
    for hc in range(NHC):
        nc.sync.dma_start(out=hidt_sb[:, hc], in_=hid_t[:, hc])
    batt_sb = singles.tile([128, NAC], f32)
    nc.sync.dma_start(out=batt_sb, in_=batt.rearrange("(ac part) -> part ac", part=128))
    wsc_sb = singles.tile([128, NAC], f32r)
    nc.sync.dma_start(out=wsc_sb, in_=wsc.rearrange("(ac part) -> part ac", part=128))
    ones_sb = singles.tile([1, 128], f32)
    nc.vector.memset(ones_sb, 1.0)

    # ---- setup: combined per-(a,b) tanh bias = hidden @ W_hid + b_att ----
    cb_sb = singles.tile([128, NAC, B_CORE], f32)
    for ac in range(NAC):
        hb_ps = ps_hb.tile([128, B_CORE], f32)
        for hc in range(NHC):
            nc.tensor.matmul(
                hb_ps,
                whid_sb[:, hc, ac * 128 : (ac + 1) * 128],
                hidt_sb[:, hc, :],
                start=(hc == 0),
                stop=(hc == NHC - 1),
            )
        nc.vector.tensor_scalar_add(
            out=cb_sb[:, ac, :], in0=hb_ps, scalar1=batt_sb[:, ac : ac + 1]
        )

    # ---- software-pipelined main loop ----
    cnn_t = [None] * NBG
    att_t = [None] * NBG
    sc_ps_t = [None] * NBG

    def stage_front(bg):
        """DMA + GEMM1 + tanh + scores for group bg."""
        b0 = bg * BG
        cnn_sb = cnn_pool.tile([128, NCC, BG, HW], f32r)
        cnn_t[bg] = cnn_sb
        src = cnn[b0 : b0 + BG].rearrange("b (cc part) p -> part b cc p", part=128)
        h = NCC // 2
        for b in range(BG):
            nc.sync.dma_start(out=cnn_sb[:, :h, b], in_=src[:, b, :h])
            nc.sync.dma_start(out=cnn_sb[:, h:, b], in_=src[:, b, h:])

        att_sb = att_pool.tile([128, NAC, COLS], f32r)
        att_t[bg] = att_sb
        for ac in range(NAC):
            att_ps = ps_att.tile([128, COLS], f32)
            for cc in range(NCC):
                nc.tensor.matmul(
                    att_ps,
                    w_sb[:, cc, ac * 128 : (ac + 1) * 128],
                    cnn_sb[:, cc].rearrange("part b p -> part (b p)"),
                    start=(cc == 0),
                    stop=(cc == NCC - 1),
                )
            for b in range(BG):
                nc.scalar.activation(
                    out=att_sb[:, ac, b * HW : (b + 1) * HW],
                    in_=att_ps[:, b * HW : (b + 1) * HW],
                    func=AF.Tanh,
                    bias=cb_sb[:, ac, b0 + b : b0 + b + 1],
                    scale=1.0,
                )
        sc_ps = ps_sc.tile([1, COLS], f32)
        sc_ps_t[bg] = sc_ps
        for ac in range(NAC):
            nc.tensor.matmul(
                sc_ps,
                wsc_sb[:, ac : ac + 1],
                att_sb[:, ac, :],
                start=(ac == 0),
                stop=(ac == NAC - 1),
            )

    def stage_back(bg):
        """softmax + weights out + broadcast + context for group bg."""
        b0 = bg * BG
        cnn_sb = cnn_t[bg]
        sc_ps = sc_ps_t[bg]

        wnorm = soft_pool.tile([1, BG, HW], f32, tag="wnorm")
        sums = soft_pool.tile([1, BG], f32, tag="sums")
        rsums = soft_pool.tile([1, BG], f32, tag="rsums")
        for b in range(BG):
            nc.scalar.activation(
                out=wnorm[0:1, b, :],
                in_=sc_ps[0:1, b * HW : (b + 1) * HW],
                func=AF.Exp,
                accum_out=sums[0:1, b : b + 1],
            )
        nc.vector.reciprocal(out=rsums, in_=sums)
        for b in range(BG):
            nc.vector.tensor_scalar_mul(
                out=wnorm[0:1, b, :],
                in0=wnorm[0:1, b, :],
                scalar1=rsums[0:1, b : b + 1],
            )
        nc.sync.dma_start(out=w_out[b0 : b0 + BG].unsqueeze(0), in_=wnorm)

        wb_ps = ps_wb.tile([128, COLS], f32)
        nc.tensor.matmul(
            wb_ps,
            ones_sb[0:1, :],
            wnorm.rearrange("o b p -> o (b p)"),
            start=True,
            stop=True,
        )

        ctx_sb = ctxc_pool.tile([128, NCC, BG], f32)
        for cc in range(NCC):
            tmp = tmp_pool.tile([128, COLS], f32)
            nc.vector.tensor_mul(
                tmp, cnn_sb[:, cc].rearrange("part b p -> part (b p)").bitcast(f32), wb_ps
            )
            junk = junk_pool.tile([128, HW], f32)
            for b in range(BG):
                nc.scalar.activation(
                    out=junk,
                    in_=tmp[:, b * HW : (b + 1) * HW],
                    func=AF.Copy,
                    accum_out=ctx_sb[:, cc, b : b + 1],
                )
        for b in range(BG):
            nc.sync.dma_start(
                out=ctx_out[b0 + b].rearrange("(cc part) -> part cc", part=128),
                in_=ctx_sb[:, :, b],
            )

    for bg in range(NBG):
        stage_front(bg)
        if bg >= 1:
            stage_back(bg - 1)
    stage_back(NBG - 1)


def build_nc():
    import concourse.tile as tile
    from concourse import bacc, mybir

    f32 = mybir.dt.float32
    f32r = mybir.dt.float32r
    nc = bacc.Bacc(target_bir_lowering=False)
    cnn = nc.dram_tensor("cnn", [B_CORE, C, HW], f32r, kind="ExternalInput")
    hid = nc.dram_tensor("hidden", [B_CORE, HID], f32r, kind="ExternalInput")
    watt = nc.dram_tensor("w_att", [C + HID, ATT], f32r, kind="ExternalInput")
    batt = nc.dram_tensor("b_att", [ATT], f32, kind="ExternalInput")
    wsc = nc.dram_tensor("w_score", [ATT], f32r, kind="ExternalInput")
    ctx_out = nc.dram_tensor("ctx_out", [B_CORE, C], f32, kind="ExternalOutput")
    w_out = nc.dram_tensor("w_out", [B_CORE, HW], f32, kind="ExternalOutput")

    with tile.TileContext(nc) as tc, ExitStack() as ctx:
        _emit(ctx, tc, cnn.ap(), hid.ap(), watt.ap(), batt.ap(), wsc.ap(), ctx_out.ap(), w_out.ap())
    nc.finalize()
    return nc


def make_in_maps(cnn_features, hidden_state, W_att, b_att, W_score):
    cnn = np.asarray(cnn_features, dtype=np.float32).reshape(B, C, HW)
    hidden = np.asarray(hidden_state, dtype=np.float32)
    watt = np.ascontiguousarray(np.asarray(W_att, dtype=np.float32))
    batt = np.ascontiguousarray(np.asarray(b_att, dtype=np.float32))
    wsc = np.ascontiguousarray(np.asarray(W_score, dtype=np.float32).reshape(ATT))
    in_maps = []
    for i in range(N_CORES):
        sl = slice(i * B_CORE, (i + 1) * B_CORE)
        in_maps.append(
            {
                "cnn": np.ascontiguousarray(cnn[sl]),
                "hidden": np.ascontiguousarray(hidden[sl]),
                "w_att": watt,
                "b_att": batt,
                "w_score": wsc,
            }
        )
    return in_maps


def kernel(cnn_features, hidden_state, W_att, b_att, W_score, b_score=None, **_):
    from concourse.bass_utils import run_bass_kernel_spmd

    nc = build_nc()
    in_maps = make_in_maps(cnn_features, hidden_state, W_att, b_att, W_score)
    res = run_bass_kernel_spmd(nc, in_maps, core_ids=list(range(N_CORES)))
    context = np.concatenate([r["ctx_out"] for r in res.results], axis=0)
    weights = np.concatenate([r["w_out"] for r in res.results], axis=0)
    return context, weights
